# revision 12
# baseline (speedup 1.0000x reference)
"""Trainium2 Bass kernel for nn_AU_Net_3573412790684 (GNN message passing).

Strategy (8 NeuronCores, SPMD + collectives):
  - Node dim padded 1026 -> NP=1152 (9*128); nodes sharded 144/core.
  - Activations kept feature-major (X^T: [features on partitions, nodes free]).
  - Big GEMMs tensor-parallel: weight column-shards per core, AllGather of
    feature-major activation slices between layers.
  - GDC exact PPR inverse via Neumann doubling on G = M^T (row-sharded):
    V = prod_j (I + G^(2^j)); 8 squarings + 8 V-updates + 9 AllGathers.
    Per-step lhsT slices produced by PE tile transposes.
  - top-128 per S-column == per partition row of V slice: 16 rounds of
    DVE max8 + match_replace; then column-normalize locally.
  - GCN layers as dense matmuls with host-built normalized adjacency AhatT.
  - matmul operands float32r (full fp32 storage; fast PE mode at N>=256).

Per-core 144-row state is stored as [128, 2*NP] SBUF tiles: block 0 =
rows 0..127 at cols [0,NP), block 1 = rows 128..143 in partitions 0..15 at
cols [NP, 2*NP).
"""
import sys
import numpy as np

sys.path.insert(0, "/opt/trn_rl_repo")
import concourse.bass as bass
from concourse import bacc
import concourse.mybir as mybir
import concourse.tile as tile
from concourse import bass_utils
from concourse.masks import make_identity

from trnutil import legalize_matmul_waits

F32 = mybir.dt.float32
F32R = mybir.dt.float32r
AF = mybir.ActivationFunctionType

# model dims
N = 1026
NP = 1152          # padded nodes (9*128)
S = NP // 8        # 144 nodes per core
DX = 4096
INS = 8192
JH = 2048
H0 = 4096
H1 = 2048
H2 = 1024
OUTS = 512
NL = 10
TOPK = 128
NSQ = 8            # factors (I+G^(2^j)) j=0..NSQ cover sum_{i<2^(NSQ+1)} M^i
NCORES = 8
NC3 = [(0, 384), (384, 384), (768, 384)]   # n-chunks of the NP free dim
BLKS = [(0, 0, 128), (1, 128, 16)]         # (block_idx, row_off, rows)


def _ceil(a, b):
    return -(-a // b)


def _mtiles(M):
    out, o = [], 0
    while o < M:
        t = min(128, M - o)
        out.append((o, t))
        o += t
    return out


class Prog:
    def __init__(self):
        self.nc = bacc.Bacc("TRN2", target_bir_lowering=False, debug=False,
                            num_devices=NCORES)
        self.uid = 0

    def name(self, p):
        self.uid += 1
        return f"{p}_{self.uid}"


def bv(t, bi, n_off=0, n_sz=NP, rows=None):
    """view of 144-row block tile: block bi, cols [n_off, n_off+n_sz)."""
    r = (128 if bi == 0 else 16) if rows is None else rows
    return t[0:r, bi * NP + n_off: bi * NP + n_off + n_sz]


def tp_gemm(P, sb, ps, kxm_srcs, kxn_srcs, M, epilogue,
            n_chunks=NC3, cache_kxm=True, tag="mm"):
    """out[M, NP] = kxm^T @ kxn via k-tile accumulation in PSUM.

    kxm_srcs: list of (dram_ap, nrows) stacked along K; each [rows, M].
    kxn_srcs: list of (dram_ap, nrows) stacked along K; each [rows, NP].
    epilogue(mi, m_off, m_sz, n_off, n_sz, psum_tile).
    """
    nc = P.nc
    ktiles = []
    for si, (ap, rows) in enumerate(kxm_srcs):
        assert rows % 128 == 0
        for r in range(0, rows, 128):
            ktiles.append((si, r))
    nkt = len(ktiles)
    rh = []
    for si, (ap, rows) in enumerate(kxn_srcs):
        for r in range(0, rows, 128):
            rh.append((si, r))
    assert len(rh) == nkt, f"kxm/kxn K mismatch {nkt} vs {len(rh)}"

    mts = _mtiles(M)

    kxm_sb = None
    if cache_kxm:
        kxm_sb = sb.tile([128, nkt * M], F32R, name=P.name("kxmC"), tag="kxmC")
        for kt, (si, r) in enumerate(ktiles):
            ap = kxm_srcs[si][0]
            nc.sync.dma_start(kxm_sb[:, kt * M:(kt + 1) * M], ap[r:r + 128, :])

    for (n_off, n_sz) in n_chunks:
        psums = []
        for mi, (m_off, m_sz) in enumerate(mts):
            psums.append(ps.tile([m_sz, n_sz], F32, name=P.name("psg"), tag=f"mm{mi}", bufs=(2 if mi < 2 else 1)))
        for kt in range(nkt):
            si, r = rh[kt]
            rt = sb.tile([128, n_sz], F32R, name=P.name("rhs"), tag="rhs", bufs=4)
            nc.sync.dma_start(rt[:], kxn_srcs[si][0][r:r + 128, n_off:n_off + n_sz])
            for mi, (m_off, m_sz) in enumerate(mts):
                if cache_kxm:
                    lh = kxm_sb[:, kt * M + m_off: kt * M + m_off + m_sz]
                else:
                    si2, r2 = ktiles[kt]
                    lht = sb.tile([128, m_sz], F32R, name=P.name("lhs"),
                                  tag="lhs", bufs=4)
                    nc.sync.dma_start(lht[:], kxm_srcs[si2][0][r2:r2 + 128,
                                                               m_off:m_off + m_sz])
                    lh = lht[:]
                nc.tensor.matmul(psums[mi][:], lh, rt[:],
                                 start=(kt == 0), stop=(kt == nkt - 1))
        for mi, (m_off, m_sz) in enumerate(mts):
            epilogue(mi, m_off, m_sz, n_off, n_sz, psums[mi])


def act_epilogue(P, sb, out_dram, bias_tile, func, out_sb_fn=None, scale=1.0):
    nc = P.nc

    def ep(mi, m_off, m_sz, n_off, n_sz, psum):
        t = sb.tile([m_sz, n_sz], F32R, name=P.name("ep"), tag="ep", bufs=3)
        if bias_tile is not None and func == AF.Copy:
            nc.vector.tensor_scalar_add(t[:], psum[:], bias_tile[0:m_sz, mi:mi + 1])
        elif bias_tile is not None:
            nc.scalar.activation(t[:], psum[:], func,
                                 bias=bias_tile[0:m_sz, mi:mi + 1], scale=scale)
        else:
            nc.scalar.activation(t[:], psum[:], func, scale=scale)
        if out_dram is not None:
            nc.sync.dma_start(out_dram[m_off:m_off + m_sz, n_off:n_off + n_sz], t[:])
        if out_sb_fn is not None:
            nc.vector.tensor_copy(out_sb_fn(mi, m_off, m_sz, n_off, n_sz), t[:])
    return ep


def load_bias(P, sb, bias_dram, M):
    nc = P.nc
    nmt = _ceil(M, 128)
    t = sb.tile([128, nmt], F32, name=P.name("bias"), tag=P.name("bias"), bufs=1)
    for mi, (m_off, m_sz) in enumerate(_mtiles(M)):
        nc.sync.dma_start(t[:m_sz, mi:mi + 1], bias_dram[m_off:m_off + m_sz, :])
    return t


def allgather(P, dr, slice_dram, full_shape, name):
    nc = P.nc
    full = dr.tile(full_shape, F32R, name=name, addr_space="Shared")
    nc.gpsimd.collective_compute(
        "AllGather", mybir.AluOpType.bypass,
        replica_groups=[list(range(NCORES))],
        ins=[slice_dram.opt()], outs=[full.opt()])
    return full


def transpose_slice(P, sb, ps, src_bt, dst_sb, ident):
    """src block-tile (144 x NP logical) -> dst [128, 9*S]: block kb holds
    (src[:, kb*128:(kb+1)*128]).T = [128 rows, 144 cols]."""
    nc = P.nc
    for kb in range(9):
        pt = ps.tile([128, 128], F32R, name=P.name("ptr"), tag="tr", bufs=2)
        nc.tensor.transpose(pt[:], bv(src_bt, 0, kb * 128, 128), ident[:])
        nc.vector.tensor_copy(dst_sb[:, kb * S: kb * S + 128], pt[:])
        pt2 = ps.tile([128, 16], F32R, name=P.name("ptr2"), tag="tr", bufs=2)
        nc.tensor.transpose(pt2[:], bv(src_bt, 1, kb * 128, 128), ident[0:16, 0:16])
        nc.vector.tensor_copy(dst_sb[:, kb * S + 128: (kb + 1) * S], pt2[:])


def dma_blocks_to_dram(P, dram_sl, src_bt):
    nc = P.nc
    nc.gpsimd.dma_start(dram_sl[0:128, :], bv(src_bt, 0))
    nc.gpsimd.dma_start(dram_sl[128:144, :], bv(src_bt, 1))


def build_program():
    P = Prog()
    nc = P.nc

    def inp(name, shape, dt=F32R):
        return nc.dram_tensor(name, shape, dt, kind="ExternalInput")

    xgT = inp("xgT", [INS, NP])
    xcol = inp("xcol", [NP, DX // 8])
    ahatT = inp("ahatT", [NP, NP])
    eyeT = inp("eyeT", [S, NP])
    vmask = inp("vmask", [1, NP], F32)
    w_jw1 = inp("w_jw1", [INS, JH // 8]); b_jb1 = inp("b_jb1", [JH // 8, 1], F32)
    w_jw2 = inp("w_jw2", [JH, S]); b_jb2 = inp("b_jb2", [S, 1], F32)
    w_ec1x = inp("w_ec1x", [DX, H0 // 8])
    w_ec1g = inp("w_ec1g", [DX, H0 // 8]); b_ec1 = inp("b_ec1", [H0 // 8, 1], F32)
    w_dr = inp("w_dr", [H0, H2 // 8]); b_dr = inp("b_dr", [H2 // 8, 1], F32)
    w_g1 = inp("w_g1", [H0, H1 // 8]); b_g1 = inp("b_g1", [H1 // 8, 1], F32)
    w_g2 = inp("w_g2", [H1, H2 // 8]); b_g2 = inp("b_g2", [H2 // 8, 1], F32)
    w_ec2 = inp("w_ec2", [H0 + H1 + H2, H2 // 8]); b_ec2 = inp("b_ec2", [H2 // 8, 1], F32)
    w_ec3 = inp("w_ec3", [H2, OUTS // 8]); b_ec3 = inp("b_ec3", [OUTS // 8, 1], F32)
    w_out = inp("w_out", [OUTS, NL]); b_out = inp("b_out", [NL, 1], F32)
    identR = inp("identR", [128, 128])
    onescol = inp("onescol", [128, 1])
    onesrow = inp("onesrow", [1, 128])

    outT = nc.dram_tensor("outT", [NL, NP], F32, kind="ExternalOutput")

    with tile.TileContext(nc) as tc:
        with tc.tile_pool(name="sb", bufs=1) as sb, \
             tc.tile_pool(name="ps", bufs=2, space="PSUM") as ps, \
             tc.tile_pool(name="dr", bufs=1, space="DRAM") as dr:

            ident = sb.tile([128, 128], F32R, name="ident")
            nc.sync.dma_start(ident[:], identR[:])

            # ============ stage A: zz1 = relu(xg @ jw1 + jb1) ============
            zz1_sl = dr.tile([JH // 8, NP], F32R, name="zz1_sl")
            bt = load_bias(P, sb, b_jb1, JH // 8)
            tp_gemm(P, sb, ps, [(w_jw1, INS)], [(xgT, INS)], JH // 8,
                    act_epilogue(P, sb, zz1_sl, bt, AF.Relu), tag="A",
                    cache_kxm=False)
            zz1_full = allgather(P, dr, zz1_sl, [JH, NP], "zz1_full")

            # ============ stage B: zz^T slice (block layout) ============
            zzT = sb.tile([128, 2 * NP], F32R, name="zzT")
            bt2 = load_bias(P, sb, b_jb2, S)

            def zz_out(mi, m_off, m_sz, n_off, n_sz):
                return bv(zzT, mi, n_off, n_sz, rows=m_sz)
            tp_gemm(P, sb, ps, [(w_jw2, JH)], [(zz1_full, JH)], S,
                    act_epilogue(P, sb, None, bt2, AF.Relu, out_sb_fn=zz_out), tag="B")

            # ============ stage C: deg / dinv ============
            ones_sl = sb.tile([128, 1], F32R, name="ones_sl")
            nc.sync.dma_start(ones_sl[:], onescol[:])
            deg_sb = sb.tile([1, NP], F32, name="deg_sb")
            for (n_off, n_sz) in NC3:
                dps = ps.tile([1, n_sz], F32, name=P.name("dps"), tag="tr")
                nc.tensor.matmul(dps[:], ones_sl[0:128, :], bv(zzT, 0, n_off, n_sz),
                                 start=True, stop=False)
                nc.tensor.matmul(dps[:], ones_sl[0:16, :], bv(zzT, 1, n_off, n_sz),
                                 start=False, stop=True)
                nc.vector.tensor_copy(deg_sb[:, n_off:n_off + n_sz], dps[:])
            deg_bin = dr.tile([1, NP], F32, name="deg_bin")
            nc.gpsimd.dma_start(deg_bin[:], deg_sb[:])
            deg_full = dr.tile([1, NP], F32, name="deg_full", addr_space="Shared")
            nc.gpsimd.collective_compute(
                "AllReduce", mybir.AluOpType.add,
                replica_groups=[list(range(NCORES))],
                ins=[deg_bin.opt()], outs=[deg_full.opt()])
            dinv_f = sb.tile([1, NP], F32, name="dinv_f")
            vm = sb.tile([1, NP], F32, name="vm")
            nc.sync.dma_start(vm[:], vmask[:])
            nc.sync.dma_start(dinv_f[:], deg_full[:])
            nc.vector.tensor_scalar_add(dinv_f[:], dinv_f[:], 1.0)
            nc.vector.reciprocal(dinv_f[:], dinv_f[:])
            nc.scalar.activation(dinv_f[:], dinv_f[:], AF.Sqrt)
            nc.vector.tensor_mul(dinv_f[:], dinv_f[:], vm[:])

            # broadcast dinv along partitions -> [128, NP]
            onesr = sb.tile([1, 128], F32R, name="onesr")
            nc.sync.dma_start(onesr[:], onesrow[:])
            dinv_fr = sb.tile([1, NP], F32R, name="dinv_fr")
            nc.vector.tensor_copy(dinv_fr[:], dinv_f[:])
            dinv_b = sb.tile([128, NP], F32R, name="dinv_b")
            for (n_off, n_sz) in NC3:
                bps = ps.tile([128, n_sz], F32, name=P.name("bps"), tag="tr")
                nc.tensor.matmul(bps[:], onesr[:], dinv_fr[:, n_off:n_off + n_sz],
                                 start=True, stop=True)
                nc.vector.tensor_copy(dinv_b[:, n_off:n_off + n_sz], bps[:])

            # eyeT blocks + per-partition dinv
            eyeT_sb = sb.tile([128, 2 * NP], F32R, name="eyeT_sb")
            nc.sync.dma_start(bv(eyeT_sb, 0), eyeT[0:128, :])
            nc.sync.dma_start(bv(eyeT_sb, 1), eyeT[128:S, :])
            dinv_p = sb.tile([128, 2], F32, name="dinv_p")
            tmpm = sb.tile([128, NP], F32R, name="tmpm", tag="scratch")
            for bi, ro, rs in BLKS:
                nc.vector.tensor_mul(tmpm[0:rs, :], bv(eyeT_sb, bi), dinv_b[0:rs, :])
                nc.vector.reduce_sum(dinv_p[0:rs, bi:bi + 1], tmpm[0:rs, :],
                                     axis=mybir.AxisListType.X)

            # ============ stage D: G slice + V init (block layout) ============
            g_sl = sb.tile([128, 2 * NP], F32R, name="g_sl0")
            v_sl = sb.tile([128, 2 * NP], F32R, name="v_sl0")
            for bi, ro, rs in BLKS:
                g = bv(g_sl, bi)
                nc.vector.tensor_add(g, bv(zzT, bi), bv(eyeT_sb, bi))
                nc.vector.tensor_scalar_mul(g, g, dinv_p[0:rs, bi:bi + 1])
                nc.vector.tensor_mul(g, g, dinv_b[0:rs, :])
                nc.vector.tensor_scalar_mul(g, g, 0.95)
                nc.vector.tensor_add(bv(v_sl, bi), bv(eyeT_sb, bi), g)

            # ============ stage E: doubling chain ============
            gT = sb.tile([128, 9 * S], F32R, name="gT")
            vT = sb.tile([128, 9 * S], F32R, name="vT")

            for j in range(1, NSQ + 2):   # j = 1..9
                last = (j == NSQ + 1)
                transpose_slice(P, sb, ps, g_sl, gT, ident)
                if j > 1:
                    transpose_slice(P, sb, ps, v_sl, vT, ident)
                gb = dr.tile([S, NP], F32R, name=P.name("g_bin"), tag="g_bin", bufs=2)
                dma_blocks_to_dram(P, gb, g_sl)
                g_full = dr.tile([NP, NP], F32R, name=P.name("g_full"),
                                 tag="g_full", bufs=2, addr_space="Shared")
                nc.gpsimd.collective_compute(
                    "AllGather", mybir.AluOpType.bypass,
                    replica_groups=[list(range(NCORES))],
                    ins=[gb.opt()], outs=[g_full.opt()])

                g_new = None if last else sb.tile([128, 2 * NP], F32R,
                                                  name=P.name("g_new"),
                                                  tag="g_new", bufs=2)
                v_new = sb.tile([128, 2 * NP], F32R, name=P.name("v_new"),
                                tag="v_new", bufs=2)
                for (n_off, n_sz) in NC3:
                    pg0 = ps.tile([128, n_sz], F32, name=P.name("pg0"), tag="mm0")
                    pg1 = ps.tile([16, n_sz], F32, name=P.name("pg1"), tag="mm1")
                    pv0 = ps.tile([128, n_sz], F32, name=P.name("pv0"), tag="mm2", bufs=1)
                    pv1 = ps.tile([16, n_sz], F32, name=P.name("pv1"), tag="mm3", bufs=1)
                    for kb in range(9):
                        rt = sb.tile([128, n_sz], F32R, name=P.name("grhs"),
                                     tag="grhs", bufs=4)
                        nc.sync.dma_start(rt[:], g_full[kb * 128:(kb + 1) * 128,
                                                        n_off:n_off + n_sz])
                        st, sp = (kb == 0), (kb == 8)
                        if not last:
                            nc.tensor.matmul(pg0[:], gT[:, kb * S: kb * S + 128],
                                             rt[:], start=st, stop=sp)
                            nc.tensor.matmul(pg1[:], gT[:, kb * S + 128:(kb + 1) * S],
                                             rt[:], start=st, stop=sp)
                        if j > 1:
                            nc.tensor.matmul(pv0[:], vT[:, kb * S: kb * S + 128],
                                             rt[:], start=st, stop=sp)
                            nc.tensor.matmul(pv1[:], vT[:, kb * S + 128:(kb + 1) * S],
                                             rt[:], start=st, stop=sp)
                    pgs = [pg0, pg1]
                    pvs = [pv0, pv1]
                    for bi, ro, rs in BLKS:
                        if not last:
                            nc.vector.tensor_copy(bv(g_new, bi, n_off, n_sz),
                                                  pgs[bi][:])
                        if j > 1:
                            nc.vector.tensor_add(bv(v_new, bi, n_off, n_sz),
                                                 bv(v_sl, bi, n_off, n_sz),
                                                 pvs[bi][:])
                if j > 1:
                    v_sl = v_new
                if not last:
                    g_sl = g_new

            # ============ stage F: topk + column normalize (on f32) ============
            vf = sb.tile([128, 2 * NP], F32, name="vf", tag="g_new", bufs=2)
            work = sb.tile([128, 2 * NP], F32, name="tkwork", tag="scratch")
            mx = sb.tile([128, 8], F32, name="tkmax")
            for bi, ro, rs in BLKS:
                nc.vector.tensor_copy(bv(vf, bi), bv(v_sl, bi))
            for bi, ro, rs in BLKS:
                cur = bv(vf, bi)
                w = bv(work, bi)
                for it in range(TOPK // 8):
                    nc.vector.max(mx[0:rs, :], cur)
                    nc.vector.match_replace(w, mx[0:rs, :], cur, 0.0)
                    cur = w
            csum = sb.tile([128, 2], F32, name="csum")
            for bi, ro, rs in BLKS:
                nc.vector.tensor_sub(bv(work, bi), bv(vf, bi), bv(work, bi))
                nc.vector.reduce_sum(csum[0:rs, bi:bi + 1], bv(work, bi),
                                     axis=mybir.AxisListType.X)
            nc.vector.tensor_scalar_add(csum[:], csum[:], 1e-30)
            nc.vector.reciprocal(csum[:], csum[:])
            for bi, ro, rs in BLKS:
                nc.vector.tensor_scalar_mul(bv(work, bi), bv(work, bi),
                                            csum[0:rs, bi:bi + 1])
            sn_bin = dr.tile([S, NP], F32R, name="sn_bin")
            dma_blocks_to_dram(P, sn_bin, work)
            snT_full = allgather(P, dr, sn_bin, [NP, NP], "snT_full")

            # ============ stage G: xn^T slice ============
            xnT_sl = dr.tile([DX // 8, NP], F32R, name="xnT_sl")
            tp_gemm(P, sb, ps, [(xcol, NP)], [(snT_full, NP)], DX // 8,
                    act_epilogue(P, sb, xnT_sl, None, AF.Copy), tag="G")
            xnT_full = allgather(P, dr, xnT_sl, [DX, NP], "xnT_full")

            # ============ stage H: ec1 two halves -> z ============
            zpart = sb.tile([128, 4 * NP], F32R, name="zpart")

            def ep_part(mi, m_off, m_sz, n_off, n_sz, psum):
                nc.vector.tensor_copy(zpart[:, mi * NP + n_off: mi * NP + n_off + n_sz],
                                      psum[:])
            tp_gemm(P, sb, ps, [(w_ec1g, DX)], [(xgT[DX:INS, :], DX)], H0 // 8,
                    ep_part, tag="Hg", cache_kxm=False)

            zT_sl = dr.tile([H0 // 8, NP], F32R, name="zT_sl")
            bt_ec1 = load_bias(P, sb, b_ec1, H0 // 8)

            def ep_z(mi, m_off, m_sz, n_off, n_sz, psum):
                t = sb.tile([m_sz, n_sz], F32R, name=P.name("epz"), tag="ep", bufs=3)
                nc.vector.tensor_add(t[:], psum[:],
                                     zpart[:, mi * NP + n_off: mi * NP + n_off + n_sz])
                nc.scalar.activation(t[:], t[:], AF.Relu,
                                     bias=bt_ec1[0:m_sz, mi:mi + 1])
                nc.sync.dma_start(zT_sl[m_off:m_off + m_sz, n_off:n_off + n_sz], t[:])
            tp_gemm(P, sb, ps, [(w_ec1x, DX)], [(xnT_full, DX)], H0 // 8,
                    ep_z, tag="Hx", cache_kxm=False)
            zT_full = allgather(P, dr, zT_sl, [H0, NP], "zT_full")

            # ============ stage I: z0 slice ============
            z0_sb = sb.tile([128, NP], F32R, name="z0_sb")
            bt_dr = load_bias(P, sb, b_dr, H2 // 8)

            def z0_out(mi, m_off, m_sz, n_off, n_sz):
                return z0_sb[0:m_sz, n_off:n_off + n_sz]
            tp_gemm(P, sb, ps, [(w_dr, H0)], [(zT_full, H0)], H2 // 8,
                    act_epilogue(P, sb, None, bt_dr, AF.Copy, out_sb_fn=z0_out),
                    tag="I")

            # ============ stage J: GCN1 ============
            W1 = H1 // 8
            h1_sb = sb.tile([128, 9 * W1], F32R, name="h1_sb")
            for mg in range(3):        # groups of 3 node-tiles
                phs = [ps.tile([128, W1], F32, name=P.name("ph"), tag=f"mm{i}",
                               bufs=(2 if i < 2 else 1)) for i in range(3)]
                for kt in range(H0 // 128):
                    wt = sb.tile([128, W1], F32R, name=P.name("wt"), tag="rhs", bufs=4)
                    nc.sync.dma_start(wt[:], w_g1[kt * 128:(kt + 1) * 128, :])
                    for i in range(3):
                        mb = mg * 3 + i
                        lz = sb.tile([128, 128], F32R, name=P.name("lz"), tag="Jl", bufs=6)
                        nc.sync.dma_start(lz[:], zT_full[kt * 128:(kt + 1) * 128,
                                                         mb * 128:(mb + 1) * 128])
                        nc.tensor.matmul(phs[i][:], lz[:], wt[:],
                                         start=(kt == 0), stop=False)
                for kt in range(H0 // 128):
                    wt = sb.tile([128, W1], F32R, name=P.name("wt"), tag="rhs", bufs=4)
                    nc.sync.dma_start(wt[:], w_g1[kt * 128:(kt + 1) * 128, :])
                    for i in range(3):
                        mb = mg * 3 + i
                        lg = sb.tile([128, 128], F32R, name=P.name("lg"), tag="Jl", bufs=6)
                        nc.sync.dma_start(lg[:], xgT[DX + kt * 128: DX + (kt + 1) * 128,
                                                     mb * 128:(mb + 1) * 128])
                        nc.tensor.matmul(phs[i][:], lg[:], wt[:],
                                         start=False, stop=(kt == H0 // 128 - 1))
                for i in range(3):
                    nc.vector.tensor_copy(h1_sb[:, (mg * 3 + i) * W1:(mg * 3 + i + 1) * W1],
                                          phs[i][:])

            z1_sl = dr.tile([H1 // 8, NP], F32R, name="z1_sl")
            bt_g1 = load_bias(P, sb, b_g1, H1 // 8)
            ep_z1 = act_epilogue(P, sb, z1_sl, bt_g1, AF.Relu)
            for (n_off, n_sz) in NC3:
                pz0 = ps.tile([128, n_sz], F32, name=P.name("pz0"), tag="mm0")
                pz1 = ps.tile([128, n_sz], F32, name=P.name("pz1"), tag="mm1")
                for kb in range(9):
                    rtt = sb.tile([128, n_sz], F32R, name=P.name("ahr"), tag="rhs", bufs=4)
                    nc.sync.dma_start(rtt[:], ahatT[kb * 128:(kb + 1) * 128,
                                                    n_off:n_off + n_sz])
                    rt = rtt[:]
                    st, sp = (kb == 0), (kb == 8)
                    nc.tensor.matmul(pz0[:], h1_sb[:, kb * W1: kb * W1 + 128], rt,
                                     start=st, stop=sp)
                    nc.tensor.matmul(pz1[:], h1_sb[:, kb * W1 + 128:(kb + 1) * W1], rt,
                                     start=st, stop=sp)
                ep_z1(0, 0, 128, n_off, n_sz, pz0)
                ep_z1(1, 128, 128, n_off, n_sz, pz1)
            z1_full = allgather(P, dr, z1_sl, [H1, NP], "z1_full")

            # ============ stage K: GCN2 ============
            W2 = H2 // 8
            h2_sb = sb.tile([128, 9 * W2], F32R, name="h2_sb")
            for mg in range(3):
                phs = [ps.tile([128, W2], F32, name=P.name("ph2"), tag=f"mm{i}",
                               bufs=(2 if i < 2 else 1)) for i in range(3)]
                for kt in range(H1 // 128):
                    wt = sb.tile([128, W2], F32R, name=P.name("wt2"), tag="rhs", bufs=4)
                    nc.sync.dma_start(wt[:], w_g2[kt * 128:(kt + 1) * 128, :])
                    for i in range(3):
                        mb = mg * 3 + i
                        lz = sb.tile([128, 128], F32R, name=P.name("lz2"), tag="Jl", bufs=6)
                        nc.sync.dma_start(lz[:], z1_full[kt * 128:(kt + 1) * 128,
                                                         mb * 128:(mb + 1) * 128])
                        nc.tensor.matmul(phs[i][:], lz[:], wt[:],
                                         start=(kt == 0), stop=(kt == H1 // 128 - 1))
                for i in range(3):
                    nc.vector.tensor_copy(h2_sb[:, (mg * 3 + i) * W2:(mg * 3 + i + 1) * W2],
                                          phs[i][:])
            z2_sl = dr.tile([H2 // 8, NP], F32R, name="z2_sl")
            bt_g2 = load_bias(P, sb, b_g2, H2 // 8)
            ep_z2 = act_epilogue(P, sb, z2_sl, bt_g2, AF.Relu)
            for (n_off, n_sz) in NC3:
                pz = ps.tile([128, n_sz], F32, name=P.name("pz2"), tag="mm0")
                for kb in range(9):
                    rtt = sb.tile([128, n_sz], F32R, name=P.name("ahr2"), tag="rhs", bufs=4)
                    nc.sync.dma_start(rtt[:], ahatT[kb * 128:(kb + 1) * 128,
                                                    n_off:n_off + n_sz])
                    nc.tensor.matmul(pz[:], h2_sb[:, kb * W2:(kb + 1) * W2], rtt[:],
                                     start=(kb == 0), stop=(kb == 8))
                ep_z2(0, 0, 128, n_off, n_sz, pz)
            z2_full = allgather(P, dr, z2_sl, [H2, NP], "z2_full")

            # ============ stage L: zc + z0 ============
            zcz0_sl = dr.tile([H2 // 8, NP], F32R, name="zcz0_sl")
            bt_ec2 = load_bias(P, sb, b_ec2, H2 // 8)

            def ep_zc(mi, m_off, m_sz, n_off, n_sz, psum):
                t = sb.tile([m_sz, n_sz], F32R, name=P.name("epc"), tag="ep", bufs=3)
                nc.scalar.activation(t[:], psum[:], AF.Relu,
                                     bias=bt_ec2[0:m_sz, mi:mi + 1])
                nc.vector.tensor_add(t[:], t[:], z0_sb[0:m_sz, n_off:n_off + n_sz])
                nc.sync.dma_start(zcz0_sl[m_off:m_off + m_sz, n_off:n_off + n_sz], t[:])
            tp_gemm(P, sb, ps,
                    [(w_ec2, H0 + H1 + H2)],
                    [(zT_full, H0), (z1_full, H1), (z2_full, H2)],
                    H2 // 8, ep_zc, tag="L")
            zcz0_full = allgather(P, dr, zcz0_sl, [H2, NP], "zcz0_full")

            # ============ stage M: zf ============
            zf_sl = dr.tile([OUTS // 8, NP], F32R, name="zf_sl")
            bt_ec3 = load_bias(P, sb, b_ec3, OUTS // 8)
            tp_gemm(P, sb, ps, [(w_ec3, H2)], [(zcz0_full, H2)], OUTS // 8,
                    act_epilogue(P, sb, zf_sl, bt_ec3, AF.Relu), tag="M")
            zf_full = allgather(P, dr, zf_sl, [OUTS, NP], "zf_full")

            # ============ stage N: out ============
            bt_out = load_bias(P, sb, b_out, NL)

            def ep_out(mi, m_off, m_sz, n_off, n_sz, psum):
                t = sb.tile([m_sz, n_sz], F32, name=P.name("epo"), tag="ep", bufs=3)
                nc.vector.tensor_scalar_add(t[:], psum[:],
                                            bt_out[0:m_sz, mi:mi + 1])
                nc.sync.dma_start(outT[m_off:m_off + m_sz, n_off:n_off + n_sz], t[:])
            tp_gemm(P, sb, ps, [(w_out, OUTS)], [(zf_full, OUTS)], NL,
                    ep_out, tag="N")

    nc.compile()
    legalize_matmul_waits(nc)
    return nc


def shard_inputs(x, gx, edge_index, jw1, jb1, jw2, jb2, ec1_w, ec1_b, dr_w, dr_b,
                 g1_w, g1_b, g2_w, g2_b, ec2_w, ec2_b, ec3_w, ec3_b, out_w, out_b):
    f32 = np.float32
    x = np.asarray(x); gx = np.asarray(gx)
    xp = np.zeros((NP, DX), f32); xp[:N] = x
    gxp = np.zeros((NP, DX), f32); gxp[:N] = gx
    xgT = np.concatenate([xp.T, gxp.T], axis=0).copy()          # [8192, NP]

    row, col = np.asarray(edge_index[0]), np.asarray(edge_index[1])
    deg = np.bincount(col, minlength=N).astype(f32) + 1.0
    dinv = (1.0 / np.sqrt(deg)).astype(f32)
    ahT = np.zeros((NP, NP), f32)
    np.add.at(ahT, (row, col), (dinv[row] * dinv[col]).astype(f32))
    ahT[np.arange(N), np.arange(N)] += dinv * dinv

    jw2p = np.zeros((JH, NP), f32); jw2p[:, :N] = jw2
    jb2p = np.zeros((NP,), f32); jb2p[:N] = jb2

    vmask = np.zeros((1, NP), f32); vmask[0, :N] = 1.0

    ins = []
    for c in range(NCORES):
        cs = slice(c * S, (c + 1) * S)
        eyeT = np.zeros((S, NP), f32)
        rr = np.arange(c * S, min((c + 1) * S, N))
        eyeT[rr - c * S, rr] = 1.0
        d = dict(
            xgT=xgT,
            xcol=xp[:, c * (DX // 8):(c + 1) * (DX // 8)],
            ahatT=ahT,
            eyeT=eyeT,
            vmask=vmask,
            w_jw1=jw1[:, c * (JH // 8):(c + 1) * (JH // 8)],
            b_jb1=np.asarray(jb1)[c * (JH // 8):(c + 1) * (JH // 8)].reshape(-1, 1),
            w_jw2=jw2p[:, cs],
            b_jb2=jb2p[cs].reshape(-1, 1),
            w_ec1x=ec1_w[:DX, c * (H0 // 8):(c + 1) * (H0 // 8)],
            w_ec1g=ec1_w[DX:, c * (H0 // 8):(c + 1) * (H0 // 8)],
            b_ec1=np.asarray(ec1_b)[c * (H0 // 8):(c + 1) * (H0 // 8)].reshape(-1, 1),
            w_dr=dr_w[:, c * (H2 // 8):(c + 1) * (H2 // 8)],
            b_dr=np.asarray(dr_b)[c * (H2 // 8):(c + 1) * (H2 // 8)].reshape(-1, 1),
            w_g1=g1_w[:, c * (H1 // 8):(c + 1) * (H1 // 8)],
            b_g1=np.asarray(g1_b)[c * (H1 // 8):(c + 1) * (H1 // 8)].reshape(-1, 1),
            w_g2=g2_w[:, c * (H2 // 8):(c + 1) * (H2 // 8)],
            b_g2=np.asarray(g2_b)[c * (H2 // 8):(c + 1) * (H2 // 8)].reshape(-1, 1),
            w_ec2=ec2_w[:, c * (H2 // 8):(c + 1) * (H2 // 8)],
            b_ec2=np.asarray(ec2_b)[c * (H2 // 8):(c + 1) * (H2 // 8)].reshape(-1, 1),
            w_ec3=ec3_w[:, c * (OUTS // 8):(c + 1) * (OUTS // 8)],
            b_ec3=np.asarray(ec3_b)[c * (OUTS // 8):(c + 1) * (OUTS // 8)].reshape(-1, 1),
            w_out=out_w,
            b_out=np.asarray(out_b).reshape(-1, 1),
            identR=np.eye(128, dtype=f32),
            onescol=np.ones((128, 1), f32),
            onesrow=np.ones((1, 128), f32),
        )
        ins.append({k: np.ascontiguousarray(v, dtype=f32) for k, v in d.items()})
    return ins


_PROG = [None]


def kernel(**inputs) -> np.ndarray:
    in_maps = shard_inputs(**inputs)
    if _PROG[0] is None:
        _PROG[0] = build_program()
    nc = _PROG[0]
    res = bass_utils.run_bass_kernel_spmd(nc, in_maps, core_ids=list(range(NCORES)))
    outT = res.results[0]["outT"]          # [10, NP]
    return np.ascontiguousarray(outT[:, :N].T)
